# revision 9
# baseline (speedup 1.0000x reference)
"""Trainium2 Bass kernel for Mixtral-style attention (B=2, S=2048, 32 q / 8 kv heads, D=128).

Sharding: 2-way data parallel over batch x 4-way tensor parallel over heads
(8 cores). Each core computes QKV projection for its head shard, RoPE, causal
GQA attention, and a partial o_proj (row-sharded). Host sums the 4 partials
per batch element.

All heavy matmuls run in bf16 with fp32 PSUM accumulation. Attention scores
are computed directly transposed (kT_blk^T @ qT_chunk) so exp(PSUM)->SBUF
lands straight in the probsT layout the attnT matmul needs; the causal mask
is a transposed-tril multiply on the diagonal 128x128 block only.

Softmax denominators stay OFF the PE streaming path: the DVE keeps a running
fp16 column-accumulator of the exp'd slab blocks, and a single ones[128,128]
matmul per (head, chunk) both partition-reduces the accumulator and
broadcasts the denominator to all 128 partitions in one 512-col pass
(replacing the per-block ones-row matmuls + reciprocal broadcast of the
previous version, ~85us of PE time). Phase B is a 3-stage software pipeline
(scores(k) | den+attnV(k-1) | epilogue(k-2)).

Phase C rotates PSUM across 6 banks and issues output-store DMA triggers
from the idle GpSimd queue so the Sync engine's ~600ns-per-DMA issue cost
never backs up the PSUM drain chain.
"""

import os
import sys

import numpy as np

for _p in ("/opt/trn_rl_repo", "/root/.axon_site/_ro/trn_rl_repo"):
    if os.path.isdir(_p) and _p not in sys.path:
        sys.path.insert(0, _p)

import ml_dtypes  # noqa: E402

import concourse.bass as bass  # noqa: E402
import concourse.mybir as mybir  # noqa: E402
import concourse.tile as tile  # noqa: E402
from concourse import bacc, bass_utils  # noqa: E402

BF16 = ml_dtypes.bfloat16
F32 = mybir.dt.float32
BF = mybir.dt.bfloat16
FP16 = mybir.dt.float16

B, S, HIDDEN = 2, 2048, 4096
NH, NKV, D = 32, 8, 128
TP, DP = 4, 2  # head-parallel x batch-parallel = 8 cores
QH = NH // TP  # 8 q heads per core
KH = NKV // TP  # 2 kv heads per core
NC_TILES = QH + 2 * KH  # 12 c-tiles of 128 per core (q..., k..., v...)
SC = 512  # s-chunk for phase A / attnT free dim
NSC = S // SC  # 4
NBLK = S // 128  # 16
ROPE_THETA = 10000.0
SM_SCALE = float(D) ** -0.5


def _emit(nc: bass.Bass):
    hT = nc.dram_tensor("hT", [128, HIDDEN // 128, S], BF, kind="ExternalInput")
    wq = nc.dram_tensor("wq", [NC_TILES, 128, 32 * 128], BF, kind="ExternalInput")
    wo = nc.dram_tensor("wo", [8, 128, 8 * 512], BF, kind="ExternalInput")
    cosT = nc.dram_tensor("cosT", [128, S], BF, kind="ExternalInput")
    sinT = nc.dram_tensor("sinT", [128, S], BF, kind="ExternalInput")
    triuD = nc.dram_tensor("triuD", [128, 128], FP16, kind="ExternalInput")
    onesD = nc.dram_tensor("onesD", [1, 128], BF, kind="ExternalInput")
    onesMD = nc.dram_tensor("onesMD", [128, 128], FP16, kind="ExternalInput")
    out = nc.dram_tensor("out", [S, HIDDEN], F32, kind="ExternalOutput")

    with tile.TileContext(nc) as tc:
        with (
            tc.tile_pool(name="const", bufs=1) as constp,
            tc.tile_pool(name="big", bufs=2) as bigp,
            tc.tile_pool(name="wt", bufs=3) as wtp,
            tc.tile_pool(name="pers", bufs=1) as pers,
            tc.tile_pool(name="rope", bufs=2) as ropep,
            tc.tile_pool(name="small", bufs=2) as smallp,
            tc.tile_pool(name="acc", bufs=2) as accp,
            tc.tile_pool(name="outp", bufs=3) as outp,
            tc.tile_pool(name="psum", bufs=2, space="PSUM") as psum,
            tc.tile_pool(name="psum_s", bufs=2, space="PSUM") as psum_s,
        ):
            # ---- startup: critical-path DMAs first, then constants ----
            ones1 = constp.tile([1, 128], BF, tag="ones1")
            nc.sync.dma_start(ones1, onesD[:])

            def load_hTc(sc):
                t = bigp.tile([128, 32, SC], BF, tag="bigslot")
                for half in range(2):
                    nc.sync.dma_start(
                        t[:, half * 16 : (half + 1) * 16, :],
                        hT[:, half * 16 : (half + 1) * 16, sc * SC : (sc + 1) * SC],
                    )
                return t

            def load_wct(c):
                t = wtp.tile([128, 32 * 128], BF, tag="wt")
                nc.sync.dma_start(t, wq[c])
                return t

            hTc0 = load_hTc(0)
            wct_pre = [load_wct(0), load_wct(1)]

            cos_sb = constp.tile([128, S], BF, tag="cos")
            sin_sb = constp.tile([128, S], BF, tag="sin")
            triu = constp.tile([128, 128], FP16, tag="triu")
            onesM = constp.tile([128, 128], FP16, tag="onesM")
            nc.sync.dma_start(cos_sb, cosT[:])
            nc.sync.dma_start(sin_sb, sinT[:])
            nc.sync.dma_start(triu, triuD[:])
            nc.sync.dma_start(onesM, onesMD[:])

            # HAM warm-up: dummy matmuls on the tiny ones row while the first
            # hidden/weight DMAs are in flight, so the PE is already
            # un-throttled when real data arrives.
            wps = psum_s.tile([128, 2, 512], F32, tag="scores")
            for w in range(32):
                nc.tensor.matmul(
                    wps[:, 0, :128], ones1, ones1, start=(w == 0), stop=(w == 31),
                    skip_group_check=True,
                )
            dwarm = smallp.tile([128, 128], BF, tag="dwarm")
            nc.scalar.copy(dwarm, wps[:, 0, :128])

            # persistent activations
            qT = pers.tile([128, QH, S], BF, tag="qT")  # [d, head, s]
            kT = pers.tile([128, KH, S], BF, tag="kT")
            vN = pers.tile([128, KH * NBLK, 128], FP16, tag="vN")  # [sk, kv*blk, d]
            aT = pers.tile([128, QH, S], BF, tag="aT")  # [d, head, s]

            def rope_into(dst, ps, sc):
                # dst = ps * cos + rot(ps) * sin ; rot = [-x2, x1]
                rot = ropep.tile([128, SC], F32, tag="rot")
                nc.scalar.mul(rot[0:64, :], ps[64:128, :], -1.0)
                nc.scalar.copy(rot[64:128, :], ps[0:64, :])
                t2 = ropep.tile([128, SC], F32, tag="t2")
                cs = cos_sb[:, sc * SC : (sc + 1) * SC]
                sn = sin_sb[:, sc * SC : (sc + 1) * SC]
                nc.vector.tensor_mul(t2, ps, cs)
                nc.vector.tensor_mul(rot, rot, sn)
                nc.vector.tensor_add(dst, t2, rot)

            # ---- Phase A: QKV^T = w_shard^T @ hidden^T, RoPE, V transpose ----
            hTc = hTc0
            hTc_next = None
            for sc in range(NSC):
                for c in range(NC_TILES):
                    if sc == 0 and c < 2:
                        wct = wct_pre[c]
                    else:
                        wct = load_wct(c)
                    if c == 2 and sc + 1 < NSC:
                        hTc_next = load_hTc(sc + 1)
                    ps = psum.tile([128, SC], F32, tag="mm512")
                    for ho in range(32):
                        nc.tensor.matmul(
                            ps,
                            wct[:, ho * 128 : (ho + 1) * 128],
                            hTc[:, ho, :],
                            start=(ho == 0),
                            stop=(ho == 31),
                        )
                    if c < QH:
                        rope_into(qT[:, c, sc * SC : (sc + 1) * SC], ps, sc)
                    elif c < QH + KH:
                        rope_into(kT[:, c - QH, sc * SC : (sc + 1) * SC], ps, sc)
                    else:
                        kv = c - QH - KH
                        vt = ropep.tile([128, SC], FP16, tag="vt")
                        nc.scalar.copy(vt, ps)
                        for j in range(SC // 128):
                            blk = sc * 4 + j
                            nc.sync.dma_start(
                                vN[:, kv * NBLK + blk, :],
                                vt[:, j * 128 : (j + 1) * 128],
                                transpose=True,
                            )
                hTc = hTc_next

            # ---- Phase B: causal GQA attention per head ----
            # slab[:, j, :] holds (unnormalized) probsT for sk-block j of the
            # current sq-chunk, in fp16. As each block is exp'd the DVE folds
            # it into a running fp16 accumulator `acc`; one ones[128,128]
            # matmul per (h, m) then partition-reduces acc AND broadcasts the
            # denominator to all 128 partitions; reciprocal + normalize are
            # DVE-only.
            def b_scores(h, m):
                # The softmax denominator is split between engines: the PE's
                # ones-matmul later streams slab blocks j < m (b_denattn) while
                # the DVE running-accumulator covers j >= m, keeping PE, ACT
                # and DVE loads balanced.
                kv = h // (QH // KH)
                slab = bigp.tile([128, NBLK, SC], FP16, tag="bigslot")
                acc = accp.tile([128, SC], FP16, tag="acc")
                qm = qT[:, h, m * 512 : (m + 1) * 512]
                for p in range(2 * m + 2):  # block pairs (2p, 2p+1)
                    j0 = 2 * p
                    diag = j0 >= 4 * m
                    pps = psum_s.tile([128, 2, 512], F32, tag="scores")
                    for u in range(2):
                        j = j0 + u
                        c0 = max(0, j - 4 * m) * 128
                        nc.tensor.matmul(
                            pps[:, u, : 512 - c0],
                            kT[:, kv, j * 128 : (j + 1) * 128],
                            qm[:, c0:],
                            start=True,
                            stop=True,
                            skip_group_check=True,
                        )
                    if not diag:
                        # fused exp over both full-width blocks
                        nc.scalar.activation(
                            slab[:, j0 : j0 + 2, :],
                            pps,
                            mybir.ActivationFunctionType.Exp,
                            scale=SM_SCALE,
                        )
                    for u in range(2):
                        j = j0 + u
                        c0 = max(0, j - 4 * m) * 128
                        if diag:
                            nc.scalar.activation(
                                slab[:, j, c0:],
                                pps[:, u, : 512 - c0],
                                mybir.ActivationFunctionType.Exp,
                                scale=SM_SCALE,
                            )
                            blk = slab[:, j, c0 : c0 + 128]
                            nc.vector.tensor_mul(blk, blk, triu)
                        if j == m:
                            nc.vector.tensor_copy(acc[:, c0:], slab[:, j, c0:])
                        elif j > m:
                            nc.vector.tensor_add(
                                acc[:, c0:], acc[:, c0:], slab[:, j, c0:]
                            )
                return slab, acc

            def b_denattn(h, m, slab, acc):
                kv = h // (QH // KH)
                dps = psum.tile([128, 512], F32, tag="mm512")
                for i in range(m):
                    nc.tensor.matmul(
                        dps, onesM, slab[:, i, :],
                        start=(i == 0), stop=False, skip_group_check=True,
                    )
                nc.tensor.matmul(
                    dps, onesM, acc, start=(m == 0), stop=True,
                    skip_group_check=True,
                )
                rcpb = smallp.tile([128, 512], F32, tag="rcpb")
                nc.vector.reciprocal_approx_fast(rcpb, dps)
                aps = psum.tile([128, 512], F32, tag="attn")
                for j in range(4 * m):
                    nc.tensor.matmul(
                        aps, vN[:, kv * NBLK + j, :], slab[:, j, :],
                        start=(j == 0), stop=False, skip_group_check=True,
                    )
                for jj in range(4):
                    j = 4 * m + jj
                    cs = slice(jj * 128, 512)
                    first = m == 0 and jj == 0
                    nc.tensor.matmul(
                        aps[:, cs], vN[:, kv * NBLK + j, :], slab[:, j, cs],
                        start=first, stop=(jj == 3), skip_group_check=True,
                    )
                return aps, rcpb

            def b_epilogue(h, m, aps, rcpb):
                nc.vector.tensor_mul(aT[:, h, m * 512 : (m + 1) * 512], aps, rcpb)

            # 3-stage software pipeline over (head, chunk): scores(k) runs on
            # PE while ACT computes exps for k and PE consumes slab(k-1);
            # epilogue(k-2) trails so its DVE chain is off the critical path.
            seq = [(h, m) for h in range(QH) for m in range(NSC)]
            st1 = st2 = None  # (h, m, slab, acc) / (h, m, aps, rcpb)
            for k, (h, m) in enumerate(seq):
                slab, acc = b_scores(h, m)
                if st1 is not None:
                    ph, pm, pslab, pacc = st1
                    st2_new = (ph, pm) + b_denattn(ph, pm, pslab, pacc)
                    if st2 is not None:
                        b_epilogue(*st2)
                    st2 = st2_new
                st1 = (h, m, slab, acc)
            ph, pm, pslab, pacc = st1
            st2_new = (ph, pm) + b_denattn(ph, pm, pslab, pacc)
            if st2 is not None:
                b_epilogue(*st2)
            b_epilogue(*st2_new)

            # ---- Phase C: partial o_proj = attnT^T @ w_o_shard ----
            # PSUM rotates over 6 banks (mm512, attn, 4x scores) so the
            # drain chain (ACT copy -> gpsimd-issued store) is never on the
            # PE's critical path.
            pps_c = None
            for hc in range(8):
                wot = wtp.tile([128, 8 * 512], BF, tag="wt")
                nc.sync.dma_start(wot, wo[hc])
                for st in range(NBLK):
                    r = st % 4
                    if r == 0:
                        ops = psum.tile([128, 512], F32, tag="mm512")
                    elif r == 1:
                        ops = psum.tile([128, 512], F32, tag="attn")
                    elif r == 2:
                        pps_c = psum_s.tile([128, 2, 512], F32, tag="scores")
                        ops = pps_c[:, 0, :]
                    else:
                        ops = pps_c[:, 1, :]
                    for cb in range(QH):
                        nc.tensor.matmul(
                            ops,
                            aT[:, cb, st * 128 : (st + 1) * 128],
                            wot[:, cb * 512 : (cb + 1) * 512],
                            start=(cb == 0),
                            stop=(cb == QH - 1),
                        )
                    ot = outp.tile([128, 512], F32, tag="ot")
                    nc.scalar.copy(ot, ops)
                    nc.gpsimd.dma_start(
                        out[st * 128 : (st + 1) * 128, hc * 512 : (hc + 1) * 512], ot
                    )

    return nc


_CACHE = {}


def build_program():
    if "nc" not in _CACHE:
        nc = bacc.Bacc()
        _emit(nc)
        nc.compile()
        _CACHE["nc"] = nc
    return _CACHE["nc"]


def host_inputs(positions, hidden_states, w_qkv, w_o):
    """Build the 8 per-core input maps (host-side shard + layout + bf16 cast)."""
    positions = np.asarray(positions)
    hidden_states = np.asarray(hidden_states, dtype=np.float32)
    w_qkv = np.asarray(w_qkv, dtype=np.float32)
    w_o = np.asarray(w_o, dtype=np.float32)

    inv_freq = 1.0 / (
        ROPE_THETA ** (np.arange(0, D, 2, dtype=np.float32) / D)
    )  # [64]
    trium = np.triu(np.ones((128, 128), dtype=np.float32)).astype(np.float16)

    # per-batch tensors
    hTs, coss, sins = [], [], []
    for b in range(B):
        hT = (
            np.ascontiguousarray(hidden_states[b].T)  # [HIDDEN, S]
            .reshape(HIDDEN // 128, 128, S)
            .transpose(1, 0, 2)  # [128, ho, S]
        )
        hTs.append(np.ascontiguousarray(hT.astype(BF16)))
        ang = positions[b].astype(np.float32)[:, None] * inv_freq[None, :]  # [S,64]
        c = np.cos(ang).T  # [64, S]
        s = np.sin(ang).T
        coss.append(np.concatenate([c, c], axis=0).astype(BF16))
        sins.append(np.concatenate([s, s], axis=0).astype(BF16))

    in_maps = []
    for core in range(8):
        b, t = divmod(core, TP)
        qcols = w_qkv[:, t * QH * D : (t + 1) * QH * D]
        kcols = w_qkv[:, NH * D + t * KH * D : NH * D + (t + 1) * KH * D]
        vcols = w_qkv[:, (NH + NKV) * D + t * KH * D : (NH + NKV) * D + (t + 1) * KH * D]
        wshard = np.concatenate([qcols, kcols, vcols], axis=1)  # [4096, 1536]
        wq_t = (
            wshard.reshape(32, 128, NC_TILES, 128)
            .transpose(2, 1, 0, 3)  # [c, p, ho, m]
            .reshape(NC_TILES, 128, 32 * 128)
            .astype(BF16)
        )
        wo_shard = w_o[t * QH * D : (t + 1) * QH * D, :]  # [1024, 4096]
        wo_t = (
            wo_shard.reshape(QH, 128, 8, 512)
            .transpose(2, 1, 0, 3)  # [hc, p, co, n]
            .reshape(8, 128, 8 * 512)
            .astype(BF16)
        )
        in_maps.append(
            {
                "hT": hTs[b],
                "wq": np.ascontiguousarray(wq_t),
                "wo": np.ascontiguousarray(wo_t),
                "cosT": coss[b],
                "sinT": sins[b],
                "triuD": trium,
                "onesD": np.ones((1, 128), dtype=BF16),
                "onesMD": np.ones((128, 128), dtype=np.float16),
            }
        )
    return in_maps


def gather_output(results):
    """Sum the 4 TP partials per batch -> [B, S, HIDDEN] fp32."""
    outs = []
    for b in range(B):
        acc = np.zeros((S, HIDDEN), dtype=np.float32)
        for t in range(TP):
            acc += results[b * TP + t]["out"]
        outs.append(acc)
    return np.stack(outs, axis=0)


def kernel(positions, hidden_states, w_qkv, w_o, trace=False):
    nc = build_program()
    in_maps = host_inputs(positions, hidden_states, w_qkv, w_o)
    last_err = None
    for attempt in range(3):
        try:
            res = bass_utils.run_bass_kernel_spmd(
                nc, in_maps, core_ids=list(range(8)), trace=trace
            )
            break
        except Exception as e:  # transient NRT/axon device errors
            last_err = e
            import time as _time

            _time.sleep(5 * (attempt + 1))
    else:
        raise last_err
    out = gather_output(res.results)
    if trace:
        kernel.last_exec_time_ns = res.exec_time_ns
        kernel.last_results = res
    return out
